# revision 19
# baseline (speedup 1.0000x reference)
"""Swin-style window-attention block (nn_Block_25718264168914) for 8x TRN2
NeuronCores. Data-parallel over batch: 4 images per core, no collectives.

v2 layout (vs baseline):
  - three phases: prepass (DMA x + LN1 stats, x resident bf16), pass A
    (attention incl LN2 stats), pass B (MLP). Scalar engine uses only Exp
    in pass A and only Gelu in pass B -> ~2 act-table loads instead of 186.
  - merged-window attention: scores per (chunk=2 windows, head) are ONE
    [32,128]x[32,98] matmul into a 4-bank psum tile; exp of all 8 heads is
    ONE scalar op; rel-pos bias applied multiplicatively (exp(bias) resident,
    zero off-diagonal) which also zeroes pad keys; AV is ONE matmul per
    head per chunk (K=128 spans both windows, block-diagonal es).
  - batched DMA: one gather per macro (8 windows = [98, 4x256]).
  - psum->sbuf copies ride on the (otherwise idle) gpsimd/Pool engine.
  - optional fp8e4 DoubleRow for fc1/fc2 (weights pre-scaled by 64).
"""

import os
import sys

sys.path.insert(0, "/opt/trn_rl_repo")

import numpy as np
import ml_dtypes

import concourse.bass as bass
import concourse.tile as tile
from concourse import mybir
from concourse.bass_utils import run_bass_kernel_spmd
from concourse.masks import make_identity

# ---------------------------------------------------------------- constants
WS = 7
NH = 8
C = 256
HD = C // NH  # 32
SCALE = HD ** -0.5
EPS = 1e-5
B, H, W = 32, 56, 56
HID = 4 * C  # 1024
N_CORES = 8
BL = B // N_CORES  # images per core
NWS = H // WS  # 8 windows per side
N_MACRO = BL * NWS  # 32 macros = (image, window-row) pairs
NT = 392  # tokens per macro
NCHUNK = 4  # chunks (2 windows / 98 tokens) per macro
TOT_CK = N_MACRO * NCHUNK  # 128 chunks per core

USE_FP8 = os.environ.get("BASS_V_FP8", "0") == "1"
WSCALE = 64.0  # fp8 weight pre-scale for fc1/fc2

F32 = mybir.dt.float32
BF16 = mybir.dt.bfloat16
FP8 = mybir.dt.float8e4
AF = mybir.ActivationFunctionType
ALU = mybir.AluOpType
DR = mybir.MatmulPerfMode.DoubleRow


def _rel_pos_index():
    coords = np.stack(
        np.meshgrid(np.arange(WS), np.arange(WS), indexing="ij"), 0
    ).reshape(2, -1)
    rel = (coords[:, :, None] - coords[:, None, :]).transpose(1, 2, 0)
    return (rel[:, :, 0] + WS - 1) * (2 * WS - 1) + (rel[:, :, 1] + WS - 1)


REL_IDX = _rel_pos_index()  # [49, 49] int


# ------------------------------------------------------- drain wait patch
# This walrus build's TPB_CTRL carries at most one sem wait; the TileContext
# tail drain waits on every touched processor. Redistribute the waits across
# single-wait NOPs emitted just before the drain.
def _install_drain_patch():
    import concourse.tile as _tile_mod
    from concourse.vector_clock import ScopedClock as _ScopedClock

    if getattr(_tile_mod.TileContext, "_drain_patch_installed", False):
        return

    def _patched(self, tick_clock, wait_clock):
        nops = [self.nc.sync.nop(nofuse=True) for _ in range(40)]
        drain_inst = self.nc.sync.drain()
        wait_clock.add_sem_waits(
            drain_inst.ins, _ScopedClock({None: tick_clock.global_clock})
        )
        si = drain_inst.ins.sync_info
        waits = list(si.on_wait) if si and si.on_wait else []
        if len(waits) > 1:
            assert len(waits) <= len(nops) + 1
            drain_inst.ins.sync_info = mybir.SyncInfo(
                on_wait=waits[:1], on_update=si.on_update or []
            )
            for nop, wt in zip(nops, waits[1:]):
                nop.ins.sync_info = mybir.SyncInfo(on_wait=[wt], on_update=[])
        self.nc.all_engine_barrier()
        assert self.sems is not None
        popped = self.nc._tile_sem_poison_stack.pop()
        assert popped is self._sem_poison
        self.nc.clear_and_free_semaphores(list(self.sems.allocated().values()))
        self.nc.all_engine_barrier()

    _tile_mod.TileContext._drain_and_barrier = _patched
    _tile_mod.TileContext._drain_patch_installed = True


# This walrus build accepts at most ONE sem wait per instruction. Tile can
# emit several (multi-producer deps). Split: insert single-wait NOPs on the
# same engine immediately before the offending instruction.
_waitnop_counter = [0]


def _split_multi_waits(nc):
    for f in nc.m.functions:
        for bb in f.blocks:
            insts = bb.instructions
            out = []
            changed = False
            for inst in insts:
                si = inst.sync_info
                waits = list(si.on_wait) if si and si.on_wait else []
                if len(waits) > 1:
                    changed = True
                    for wt in waits[:-1]:
                        _waitnop_counter[0] += 1
                        nop = mybir.InstNoOp(
                            name=f"I-waitsplit-{_waitnop_counter[0]}",
                            ins=[],
                            outs=[],
                        )
                        nop.engine = inst.engine
                        nop.sync_info = mybir.SyncInfo(on_wait=[wt], on_update=[])
                        try:
                            nc.register_instruction(nop, overwrite=True)
                        except Exception:
                            pass
                        out.append(nop)
                    inst.sync_info = mybir.SyncInfo(
                        on_wait=[waits[-1]], on_update=si.on_update or []
                    )
                out.append(inst)
            if changed:
                bb.instructions = out


# ------------------------------------------------------------ bass program
def build_program(use_fp8=USE_FP8, has_bq=True, has_b1=True):
    _install_drain_patch()
    nc = bass.Bass()
    MDT = FP8 if use_fp8 else BF16  # dtype for fc1/fc2 operands

    d_x = nc.dram_tensor("x", [BL, H, W, C], F32, kind="ExternalInput")
    d_wq = nc.dram_tensor("wq", [128, 2, C], BF16, kind="ExternalInput")
    d_wk = nc.dram_tensor("wk", [128, 2, C], BF16, kind="ExternalInput")
    d_wv = nc.dram_tensor("wv", [128, 2, C], BF16, kind="ExternalInput")
    d_wp = nc.dram_tensor("wp", [128, 2, C], BF16, kind="ExternalInput")
    d_w1 = nc.dram_tensor("w1", [128, 2, HID], MDT, kind="ExternalInput")
    d_w2 = nc.dram_tensor("w2", [128, 8, C], MDT, kind="ExternalInput")
    d_bq = nc.dram_tensor("bq", [128, 2], F32, kind="ExternalInput")
    d_bk = nc.dram_tensor("bk", [128, 2], F32, kind="ExternalInput")
    d_b1 = nc.dram_tensor("b1", [128, 8], F32, kind="ExternalInput")
    d_expb = nc.dram_tensor("expb", [128, 4, 2, 98], BF16, kind="ExternalInput")
    d_out = nc.dram_tensor("out", [BL, H, W, C], F32, kind="ExternalOutput")

    # one macro = one (image, window-row): [98 tokens(w,r,t), 4 chunks, C]
    xw6 = d_x.rearrange(
        "b (wr r) (ck w t) ch -> (b wr) r ck w t ch", r=WS, ck=NCHUNK, w=2, t=WS
    )
    ow6 = d_out.rearrange(
        "b (wr r) (ck w t) ch -> (b wr) r ck w t ch", r=WS, ck=NCHUNK, w=2, t=WS
    )

    def xv(mt, w, ck):
        # [7r, 7t, 256ch]; (t, ch) merge -> 2-dim DRAM AP
        return xw6[mt][:, ck, w, :, :]

    def ov(mt, w, ck):
        return ow6[mt][:, ck, w, :, :]

    from contextlib import ExitStack

    with tile.TileContext(nc) as tc:
        with ExitStack() as octx:
            resident = octx.enter_context(tc.tile_pool(name="res", bufs=1))
            # ------------- residents
            wq_sb = resident.tile([128, 2, C], BF16)
            nc.sync.dma_start(wq_sb, d_wq[:])
            wk_sb = resident.tile([128, 2, C], BF16)
            nc.sync.dma_start(wk_sb, d_wk[:])
            wv_sb = resident.tile([128, 2, C], BF16)
            nc.sync.dma_start(wv_sb, d_wv[:])
            wp_sb = resident.tile([128, 2, C], BF16)
            nc.sync.dma_start(wp_sb, d_wp[:])
            w1_sb = resident.tile([128, 2, HID], MDT)
            nc.sync.dma_start(w1_sb, d_w1[:])
            w2_sb = resident.tile([128, 8, C], MDT)
            nc.sync.dma_start(w2_sb, d_w2[:])
            bq_sb = resident.tile([128, 2], F32)
            nc.sync.dma_start(bq_sb, d_bq[:])
            bk_sb = resident.tile([128, 2], F32)
            nc.sync.dma_start(bk_sb, d_bk[:])
            b1_sb = resident.tile([128, 8], F32)
            nc.sync.dma_start(b1_sb, d_b1[:])
            expb_sb = resident.tile([128, 4, 2, 98], BF16)
            nc.sync.dma_start(expb_sb, d_expb[:])
            ident = resident.tile([128, 128], BF16)
            make_identity(nc, ident)
            eps_sb = resident.tile([128, 1], F32)
            nc.vector.memset(eps_sb, EPS)

            # big residents: x (becomes x2 in place), LN stats
            x_res = resident.tile([98, TOT_CK, C], BF16)
            mv1 = resident.tile([98, TOT_CK, 2], F32)
            rstd1 = resident.tile([98, TOT_CK], F32)
            mv2 = resident.tile([98, TOT_CK, 2], F32)
            rstd2 = resident.tile([98, TOT_CK], F32)

            # =========================== PREPASS ===========================
            with ExitStack() as ctx:
                xin = ctx.enter_context(tc.tile_pool(name="xin", bufs=3))
                stp = ctx.enter_context(tc.tile_pool(name="stp", bufs=4))
                for mt in range(N_MACRO):
                    xst = xin.tile([98, NCHUNK, 264], F32, tag="xst")
                    for ck in range(NCHUNK):
                        for w in range(2):
                            nc.sync.dma_start(
                                xst[w * 49 : (w + 1) * 49, ck, 0:C], xv(mt, w, ck)
                            )
                    # cast to resident bf16 (scalar engine, table-free Copy)
                    nc.scalar.activation(
                        x_res[:, mt * 4 : (mt + 1) * 4, :],
                        xst[:, :, 0:C],
                        AF.Copy,
                    )
                    for ck in range(NCHUNK):
                        st = stp.tile([98, 6], F32, tag="st")
                        nc.vector.bn_stats(st, xst[:, ck, 0:C])
                        nc.vector.bn_aggr(mv1[:, mt * 4 + ck, :], st)
                    if mt % 8 == 7:
                        g0 = (mt - 7) * 4
                        sd1 = stp.tile([98, 32], F32, tag="sd1")
                        nc.scalar.activation(
                            sd1, mv1[:, g0 : g0 + 32, 1], AF.Sqrt, bias=eps_sb[:98]
                        )
                        nc.vector.reciprocal(rstd1[:, g0 : g0 + 32], sd1)

            # =========================== PASS A ============================
            with ExitStack() as ctx:
                xhp = ctx.enter_context(tc.tile_pool(name="xhp", bufs=4))
                hTp = ctx.enter_context(tc.tile_pool(name="hTp", bufs=2))
                qTp = ctx.enter_context(tc.tile_pool(name="qTp", bufs=2))
                kTp = ctx.enter_context(tc.tile_pool(name="kTp", bufs=2))
                vap = ctx.enter_context(tc.tile_pool(name="vap", bufs=8))
                esp = ctx.enter_context(tc.tile_pool(name="esp", bufs=4))
                atp = ctx.enter_context(tc.tile_pool(name="atp", bufs=4))
                aTp = ctx.enter_context(tc.tile_pool(name="aTp", bufs=2))
                sta = ctx.enter_context(tc.tile_pool(name="sta", bufs=4))
                ps_sc = ctx.enter_context(
                    tc.tile_pool(name="ps_sc", bufs=1, space="PSUM")
                )
                ps_av = ctx.enter_context(
                    tc.tile_pool(name="ps_av", bufs=1, space="PSUM")
                )
                ps_mm = ctx.enter_context(
                    tc.tile_pool(name="ps_mm", bufs=1, space="PSUM")
                )
                ps_tr = ctx.enter_context(
                    tc.tile_pool(name="ps_tr", bufs=1, space="PSUM")
                )

                for mt in range(N_MACRO):
                    c0 = mt * 4
                    # ---- LN1 apply + transpose -> hT [128, 2, 392] bf16
                    hT = hTp.tile([128, 2, NT], BF16, tag="hT")
                    for ck in range(NCHUNK):
                        xh = xhp.tile([98, C], BF16, tag="xh")
                        nc.gpsimd.tensor_scalar(
                            xh,
                            x_res[:, c0 + ck, :],
                            mv1[:, c0 + ck, 0:1],
                            rstd1[:, c0 + ck : c0 + ck + 1],
                            ALU.subtract,
                            ALU.mult,
                        )
                        pst = ps_tr.tile(
                            [128, 2, 98], BF16, tag="pst",
                            padded_shape=[128, 2, 1024],
                        )
                        for kc in range(2):
                            nc.tensor.transpose(
                                pst[:, kc, :], xh[:, kc * 128 : (kc + 1) * 128],
                                ident[:98, :98],
                            )
                        nc.scalar.copy(hT[:, :, ck * 98 : (ck + 1) * 98], pst)

                    # ---- q, k channel-major
                    qT = qTp.tile([128, 2, NT], BF16, tag="qT")
                    kT = kTp.tile([128, 2, NCHUNK, 128], BF16, tag="kT")
                    if mt < 2:  # zero pad key slots once per buffer
                        nc.vector.memset(kT, 0.0)
                    for g in range(2):
                        psq = ps_mm.tile([128, NT], F32, tag="mm")
                        for kc in range(2):
                            nc.tensor.matmul(
                                psq,
                                lhsT=wq_sb[:, kc, g * 128 : (g + 1) * 128],
                                rhs=hT[:, kc, :],
                                start=(kc == 0),
                                stop=(kc == 1),
                            )
                        if has_bq:
                            nc.vector.tensor_scalar_add(
                                qT[:, g, :], psq, bq_sb[:, g : g + 1]
                            )
                        else:
                            nc.vector.tensor_copy(qT[:, g, :], psq)
                        psk = ps_mm.tile([128, NT], F32, tag="mm")
                        for kc in range(2):
                            nc.tensor.matmul(
                                psk,
                                lhsT=wk_sb[:, kc, g * 128 : (g + 1) * 128],
                                rhs=hT[:, kc, :],
                                start=(kc == 0),
                                stop=(kc == 1),
                            )
                        pskv = psk.rearrange("p (ck wt) -> p ck wt", ck=NCHUNK)
                        for w in range(2):
                            dst = kT[:, g, :, w * 64 : w * 64 + 49]
                            src = pskv[:, :, w * 49 : (w + 1) * 49]
                            if has_bq:
                                nc.vector.tensor_scalar_add(
                                    dst, src, bk_sb[:, g : g + 1]
                                )
                            else:
                                nc.vector.tensor_copy(dst, src)

                    # ---- per chunk: v, scores, exp*bias, AV, normalize
                    a_tms = []
                    for ck in range(NCHUNK):
                        # v token-major -> va [128 jslot, 8, 33]
                        psv = ps_mm.tile([128, C], F32, tag="mm", name="psv")
                        for w in range(2):
                            t0 = ck * 98 + w * 49
                            for kc in range(2):
                                nc.tensor.matmul(
                                    psv[w * 64 : w * 64 + 49, :],
                                    lhsT=hT[:, kc, t0 : t0 + 49],
                                    rhs=wv_sb[:, kc, :],
                                    start=(kc == 0),
                                    stop=(kc == 1),
                                )
                        va = vap.tile([128, NH, HD + 1], BF16, tag="va")
                        if mt < 2:  # preset ones column + zero pad rows, per slot
                            nc.gpsimd.memset(va, 0.0)
                            nc.gpsimd.memset(va[:, :, HD : HD + 1], 1.0)
                        psvv = psv.rearrange("p (h d) -> p h d", h=NH)
                        nc.vector.tensor_copy(va[0:49, :, 0:HD], psvv[0:49])
                        nc.vector.tensor_copy(va[64:113, :, 0:HD], psvv[64:113])

                        # scores: [128, 4r, 2*98] psum (4 banks), K=32 packed
                        pss = ps_sc.tile(
                            [128, 4, 196], F32, tag="sc", padded_shape=[128, 4, 512]
                        )
                        for r in range(4):
                            for g in range(2):
                                nc.tensor.matmul(
                                    pss[:, r, g * 98 : (g + 1) * 98],
                                    lhsT=kT[r * 32 : (r + 1) * 32, g, ck, :],
                                    rhs=qT[r * 32 : (r + 1) * 32, g, ck * 98 : ck * 98 + 98],
                                    start=True,
                                    stop=True,
                                    tile_position=(r * 32, 0),
                                    skip_group_check=True,
                                )
                        es = esp.tile([128, 4, 2, 98], BF16, tag="es")
                        es_f = es.rearrange("p r g i -> p r (g i)")
                        nc.scalar.activation(es_f, pss, AF.Exp)
                        # multiplicative rel-pos bias; zeroes pad keys and
                        # cross-window blocks
                        nc.gpsimd.tensor_tensor(es, es, expb_sb, ALU.mult)

                        # AV: one matmul per head, K=128 covers both windows
                        psav = ps_av.tile([98, NH, HD + 1], F32, tag="av")
                        for h in range(NH):
                            r, g = h % 4, h // 4
                            nc.tensor.matmul(
                                psav[:, h, :],
                                lhsT=es[:, r, g, :],
                                rhs=va[:, h, :],
                                start=True,
                                stop=True,
                            )
                        rd = sta.tile([98, NH, 1], F32, tag="rd")
                        nc.vector.reciprocal(rd, psav[:, :, HD : HD + 1])
                        a_tm = atp.tile([98, NH, HD], BF16, tag="atm")
                        nc.vector.tensor_tensor(
                            a_tm,
                            psav[:, :, 0:HD],
                            rd.to_broadcast([98, NH, HD]),
                            ALU.mult,
                        )
                        a_tms.append(a_tm)

                    # ---- transpose attn out, proj, residual, LN2 stats
                    aT = aTp.tile([128, 2, NT], BF16, tag="aT")
                    for ck in range(NCHUNK):
                        af = a_tms[ck].rearrange("p h d -> p (h d)")
                        psat = ps_tr.tile(
                            [128, 2, 98], BF16, tag="pst",
                            padded_shape=[128, 2, 1024],
                        )
                        for kc in range(2):
                            nc.tensor.transpose(
                                psat[:, kc, :], af[:, kc * 128 : (kc + 1) * 128],
                                ident[:98, :98],
                            )
                        nc.scalar.copy(aT[:, :, ck * 98 : (ck + 1) * 98], psat)
                    for ck in range(NCHUNK):
                        psp = ps_mm.tile([98, C], F32, tag="mm", name="psp")
                        for kc in range(2):
                            nc.tensor.matmul(
                                psp,
                                lhsT=aT[:, kc, ck * 98 : (ck + 1) * 98],
                                rhs=wp_sb[:, kc, :],
                                start=(kc == 0),
                                stop=(kc == 1),
                            )
                        # x2 = x + proj(attn)  (in place into x_res)
                        nc.vector.tensor_tensor(
                            x_res[:, c0 + ck, :], psp, x_res[:, c0 + ck, :], ALU.add
                        )
                        st2 = sta.tile([98, 6], F32, tag="st2")
                        nc.vector.bn_stats(st2, x_res[:, c0 + ck, :])
                        nc.vector.bn_aggr(mv2[:, c0 + ck, :], st2)
                    if mt % 8 == 7:
                        g0 = (mt - 7) * 4
                        sd2 = sta.tile([98, 32], F32, tag="sd2")
                        nc.scalar.activation(
                            sd2, mv2[:, g0 : g0 + 32, 1], AF.Sqrt, bias=eps_sb[:98]
                        )
                        nc.vector.reciprocal(rstd2[:, g0 : g0 + 32], sd2)

            # =========================== PASS B ============================
            with ExitStack() as ctx:
                xh2p = ctx.enter_context(tc.tile_pool(name="xh2p", bufs=4))
                h2Tp = ctx.enter_context(tc.tile_pool(name="h2Tp", bufs=2))
                gTp = ctx.enter_context(tc.tile_pool(name="gTp", bufs=2))
                otp = ctx.enter_context(tc.tile_pool(name="otp", bufs=2))
                ps_f1 = ctx.enter_context(
                    tc.tile_pool(name="ps_f1", bufs=1, space="PSUM")
                )
                ps_f2 = ctx.enter_context(
                    tc.tile_pool(name="ps_f2", bufs=2, space="PSUM")
                )
                ps_t2 = ctx.enter_context(
                    tc.tile_pool(name="ps_t2", bufs=1, space="PSUM")
                )

                for mt in range(N_MACRO):
                    c0 = mt * 4
                    h2T = h2Tp.tile([128, 2, NT], MDT, tag="h2T")
                    for ck in range(NCHUNK):
                        xh2 = xh2p.tile([98, C], BF16, tag="xh2")
                        nc.gpsimd.tensor_scalar(
                            xh2,
                            x_res[:, c0 + ck, :],
                            mv2[:, c0 + ck, 0:1],
                            rstd2[:, c0 + ck : c0 + ck + 1],
                            ALU.subtract,
                            ALU.mult,
                        )
                        pst2 = ps_t2.tile(
                            [128, 2, 98], BF16, tag="pst2",
                            padded_shape=[128, 2, 1024],
                        )
                        for kc in range(2):
                            nc.tensor.transpose(
                                pst2[:, kc, :], xh2[:, kc * 128 : (kc + 1) * 128],
                                ident[:98, :98],
                            )
                        nc.vector.tensor_copy(
                            h2T[:, :, ck * 98 : (ck + 1) * 98], pst2
                        )

                    # fc1 + gelu -> gT [128, 8(mc), 392]
                    gT = gTp.tile([128, 2, 4, NT], MDT, tag="gT")
                    for mq in range(2):
                        psf = ps_f1.tile(
                            [128, 4, NT], F32, tag="f1", padded_shape=[128, 4, 512]
                        )
                        for sub in range(4):
                            mc = mq * 4 + sub
                            if use_fp8:
                                nc.tensor.matmul(
                                    psf[:, sub, :],
                                    lhsT=w1_sb[:, :, mc * 128 : (mc + 1) * 128],
                                    rhs=h2T,
                                    start=True,
                                    stop=True,
                                    perf_mode=DR,
                                )
                            else:
                                for kc in range(2):
                                    nc.tensor.matmul(
                                        psf[:, sub, :],
                                        lhsT=w1_sb[:, kc, mc * 128 : (mc + 1) * 128],
                                        rhs=h2T[:, kc, :],
                                        start=(kc == 0),
                                        stop=(kc == 1),
                                    )
                        inv = (1.0 / WSCALE) if use_fp8 else 1.0
                        if has_b1:
                            # per-mc bias varies along free dim -> per-mc ops
                            for sub in range(4):
                                nc.scalar.activation(
                                    gT[:, mq, sub, :],
                                    psf[:, sub, :],
                                    AF.Gelu,
                                    bias=b1_sb[:, mq * 4 + sub : mq * 4 + sub + 1],
                                    scale=inv,
                                )
                        else:
                            nc.scalar.activation(gT[:, mq], psf, AF.Gelu, scale=inv)

                    # fc2 + residual -> out
                    o_t = otp.tile([98, NCHUNK, 264], F32, tag="ot")
                    for ck in range(NCHUNK):
                        ps2 = ps_f2.tile([98, C], F32, tag="f2")
                        if use_fp8:
                            for pr in range(4):
                                nc.tensor.matmul(
                                    ps2,
                                    lhsT=gT[:, pr // 2, (pr % 2) * 2 : (pr % 2) * 2 + 2,
                                            ck * 98 : (ck + 1) * 98],
                                    rhs=w2_sb[:, 2 * pr : 2 * pr + 2, :],
                                    start=(pr == 0),
                                    stop=(pr == 3),
                                    perf_mode=DR,
                                )
                        else:
                            for hc in range(8):
                                nc.tensor.matmul(
                                    ps2,
                                    lhsT=gT[:, hc // 4, hc % 4, ck * 98 : (ck + 1) * 98],
                                    rhs=w2_sb[:, hc, :],
                                    start=(hc == 0),
                                    stop=(hc == 7),
                                )
                        if use_fp8:
                            nc.vector.scalar_tensor_tensor(
                                o_t[:, ck, 0:C],
                                ps2,
                                1.0 / WSCALE,
                                x_res[:, c0 + ck, :],
                                ALU.mult,
                                ALU.add,
                            )
                        else:
                            nc.vector.tensor_tensor(
                                o_t[:, ck, 0:C], ps2, x_res[:, c0 + ck, :], ALU.add
                            )
                    for ck in range(NCHUNK):
                        for w in range(2):
                            nc.sync.dma_start(
                                ov(mt, w, ck), o_t[w * 49 : (w + 1) * 49, ck, 0:C]
                            )

    _split_multi_waits(nc)
    return nc


# ------------------------------------------------------------- host wrapper
_PROGRAM_CACHE = {}


def _prep_weights(norm1_g, norm1_b, qkv_w, qkv_b, bias_table, proj_w, proj_b,
                  norm2_g, norm2_b, fc1_w, fc1_b, fc2_w, fc2_b, use_fp8=USE_FP8):
    f32 = np.float32
    bf16 = ml_dtypes.bfloat16
    fp8 = ml_dtypes.float8_e4m3fn
    # fold LN1 affine into qkv weights
    wqkv = (norm1_g[:, None] * qkv_w).astype(f32)  # [C, 3C]
    bqkv = (norm1_b @ qkv_w + qkv_b).astype(f32)  # [3C]
    wq = wqkv[:, 0:C] * SCALE
    bq = bqkv[0:C] * SCALE
    wk = wqkv[:, C : 2 * C]
    bk = bqkv[C : 2 * C]
    wv = wqkv[:, 2 * C : 3 * C]
    bv = bqkv[2 * C : 3 * C]
    # fold LN2 affine into fc1
    w1 = (norm2_g[:, None] * fc1_w).astype(f32)  # [C, HID]
    b1 = (norm2_b @ fc1_w + fc1_b).astype(f32)  # [HID]

    def kpart(w):  # [K, O] -> [128, K//128, O]
        k, o = w.shape
        return np.ascontiguousarray(
            w.reshape(k // 128, 128, o).transpose(1, 0, 2)
        )

    mdt = fp8 if use_fp8 else bf16
    ws = WSCALE if use_fp8 else 1.0
    arrs = {
        "wq": kpart(wq).astype(bf16),
        "wk": kpart(wk).astype(bf16),
        "wv": kpart(wv).astype(bf16),
        "wp": kpart(proj_w.astype(f32)).astype(bf16),
        "w1": kpart(w1 * ws).astype(mdt),
        "w2": kpart(fc2_w.astype(f32) * ws).astype(mdt),
        "bq": np.ascontiguousarray(bq.reshape(2, 128).T).astype(f32),
        "bk": np.ascontiguousarray(bk.reshape(2, 128).T).astype(f32),
        "b1": np.ascontiguousarray((b1 / 1.0).reshape(8, 128).T).astype(f32),
    }
    # multiplicative rel-pos bias table: expb[j, r, g, i]; zero on pad key
    # rows and cross-window blocks
    bias_full = np.asarray(bias_table)[REL_IDX]  # [i, j, NH]
    eb = np.zeros((128, 4, 2, 98), dtype=f32)
    for h in range(NH):
        r, g = h % 4, h // 4
        bj = np.exp(bias_full[:, :, h].T.astype(f32))  # [j, i]
        eb[0:49, r, g, 0:49] = bj
        eb[64:113, r, g, 49:98] = bj
    arrs["expb"] = eb.astype(bf16)

    for name, v in (("bv", bv), ("bp", proj_b), ("b2", fc2_b)):
        assert np.abs(v).max() < 1e-30, f"nonzero {name} not supported yet"
    has_bq = bool(np.abs(bq).max() > 0 or np.abs(bk).max() > 0)
    has_b1 = bool(np.abs(b1).max() > 0)
    return arrs, has_bq, has_b1


def kernel(**inputs):
    x = np.asarray(inputs["x"], dtype=np.float32)
    prep, has_bq, has_b1 = _prep_weights(
        np.asarray(inputs["norm1_g"], np.float32),
        np.asarray(inputs["norm1_b"], np.float32),
        np.asarray(inputs["qkv_w"], np.float32),
        np.asarray(inputs["qkv_b"], np.float32),
        np.asarray(inputs["bias_table"], np.float32),
        np.asarray(inputs["proj_w"], np.float32),
        np.asarray(inputs["proj_b"], np.float32),
        np.asarray(inputs["norm2_g"], np.float32),
        np.asarray(inputs["norm2_b"], np.float32),
        np.asarray(inputs["fc1_w"], np.float32),
        np.asarray(inputs["fc1_b"], np.float32),
        np.asarray(inputs["fc2_w"], np.float32),
        np.asarray(inputs["fc2_b"], np.float32),
    )

    key = ("nc", USE_FP8, has_bq, has_b1)
    if key not in _PROGRAM_CACHE:
        _PROGRAM_CACHE[key] = build_program(USE_FP8, has_bq, has_b1)
        _PROGRAM_CACHE["nc"] = _PROGRAM_CACHE[key]
    nc = _PROGRAM_CACHE[key]

    in_maps = []
    for c in range(N_CORES):
        m = dict(prep)
        m["x"] = np.ascontiguousarray(x[c * BL : (c + 1) * BL])
        in_maps.append(m)

    res = run_bass_kernel_spmd(nc, in_maps, core_ids=list(range(N_CORES)))
    out = np.concatenate([res.results[c]["out"] for c in range(N_CORES)], axis=0)
    return out.astype(np.float32)


# revision 22
# speedup vs baseline: 1.5329x; 1.5329x over previous
"""Swin-style window-attention block (nn_Block_25718264168914) for 8x TRN2
NeuronCores. Data-parallel over batch: 4 images per core, no collectives.

v2 layout (vs baseline):
  - three phases: prepass (DMA x + LN1 stats, x resident bf16), pass A
    (attention incl LN2 stats), pass B (MLP). Scalar engine uses only Exp
    in pass A and only Gelu in pass B -> ~2 act-table loads instead of 186.
  - merged-window attention: scores per (chunk=2 windows, head) are ONE
    [32,128]x[32,98] matmul into a 4-bank psum tile; exp of all 8 heads is
    ONE scalar op; rel-pos bias applied multiplicatively (exp(bias) resident,
    zero off-diagonal) which also zeroes pad keys; AV is ONE matmul per
    head per chunk (K=128 spans both windows, block-diagonal es).
  - batched DMA: one gather per macro (8 windows = [98, 4x256]).
  - psum->sbuf copies ride on the (otherwise idle) gpsimd/Pool engine.
  - optional fp8e4 DoubleRow for fc1/fc2 (weights pre-scaled by 64).
"""

import os
import sys

sys.path.insert(0, "/opt/trn_rl_repo")

import numpy as np
import ml_dtypes

import concourse.bass as bass
import concourse.tile as tile
from concourse import mybir
from concourse.bass_utils import run_bass_kernel_spmd
from concourse.masks import make_identity

# ---------------------------------------------------------------- constants
WS = 7
NH = 8
C = 256
HD = C // NH  # 32
SCALE = HD ** -0.5
EPS = 1e-5
B, H, W = 32, 56, 56
HID = 4 * C  # 1024
N_CORES = 8
BL = B // N_CORES  # images per core
NWS = H // WS  # 8 windows per side
N_MACRO = BL * NWS  # 32 macros = (image, window-row) pairs
NT = 392  # tokens per macro
NCHUNK = 4  # chunks (2 windows / 98 tokens) per macro
TOT_CK = N_MACRO * NCHUNK  # 128 chunks per core

USE_FP8 = os.environ.get("BASS_V_FP8", "0") == "1"
WSCALE = 64.0  # fp8 weight pre-scale for fc1/fc2

F32 = mybir.dt.float32
BF16 = mybir.dt.bfloat16
FP8 = mybir.dt.float8e4
AF = mybir.ActivationFunctionType
ALU = mybir.AluOpType
DR = mybir.MatmulPerfMode.DoubleRow


def _rel_pos_index():
    coords = np.stack(
        np.meshgrid(np.arange(WS), np.arange(WS), indexing="ij"), 0
    ).reshape(2, -1)
    rel = (coords[:, :, None] - coords[:, None, :]).transpose(1, 2, 0)
    return (rel[:, :, 0] + WS - 1) * (2 * WS - 1) + (rel[:, :, 1] + WS - 1)


REL_IDX = _rel_pos_index()  # [49, 49] int


# ------------------------------------------------------- drain wait patch
# This walrus build's TPB_CTRL carries at most one sem wait; the TileContext
# tail drain waits on every touched processor. Redistribute the waits across
# single-wait NOPs emitted just before the drain.
def _install_drain_patch():
    import concourse.tile as _tile_mod
    from concourse.vector_clock import ScopedClock as _ScopedClock

    if getattr(_tile_mod.TileContext, "_drain_patch_installed", False):
        return

    def _patched(self, tick_clock, wait_clock):
        nops = [self.nc.sync.nop(nofuse=True) for _ in range(40)]
        drain_inst = self.nc.sync.drain()
        wait_clock.add_sem_waits(
            drain_inst.ins, _ScopedClock({None: tick_clock.global_clock})
        )
        si = drain_inst.ins.sync_info
        waits = list(si.on_wait) if si and si.on_wait else []
        if len(waits) > 1:
            assert len(waits) <= len(nops) + 1
            drain_inst.ins.sync_info = mybir.SyncInfo(
                on_wait=waits[:1], on_update=si.on_update or []
            )
            for nop, wt in zip(nops, waits[1:]):
                nop.ins.sync_info = mybir.SyncInfo(on_wait=[wt], on_update=[])
        self.nc.all_engine_barrier()
        assert self.sems is not None
        popped = self.nc._tile_sem_poison_stack.pop()
        assert popped is self._sem_poison
        self.nc.clear_and_free_semaphores(list(self.sems.allocated().values()))
        self.nc.all_engine_barrier()

    _tile_mod.TileContext._drain_and_barrier = _patched
    _tile_mod.TileContext._drain_patch_installed = True


# This walrus build accepts at most ONE sem wait per instruction. Tile can
# emit several (multi-producer deps). Split: insert single-wait NOPs on the
# same engine immediately before the offending instruction.
_waitnop_counter = [0]


def _split_multi_waits(nc):
    for f in nc.m.functions:
        for bb in f.blocks:
            insts = bb.instructions
            out = []
            changed = False
            for inst in insts:
                si = inst.sync_info
                waits = list(si.on_wait) if si and si.on_wait else []
                if len(waits) > 1:
                    changed = True
                    for wt in waits[:-1]:
                        _waitnop_counter[0] += 1
                        nop = mybir.InstNoOp(
                            name=f"I-waitsplit-{_waitnop_counter[0]}",
                            ins=[],
                            outs=[],
                        )
                        nop.engine = inst.engine
                        nop.sync_info = mybir.SyncInfo(on_wait=[wt], on_update=[])
                        try:
                            nc.register_instruction(nop, overwrite=True)
                        except Exception:
                            pass
                        out.append(nop)
                    inst.sync_info = mybir.SyncInfo(
                        on_wait=[waits[-1]], on_update=si.on_update or []
                    )
                out.append(inst)
            if changed:
                bb.instructions = out


# ------------------------------------------------------------ bass program
def build_program(use_fp8=USE_FP8, has_bq=True, has_b1=True):
    _install_drain_patch()
    nc = bass.Bass()
    MDT = FP8 if use_fp8 else BF16  # dtype for fc1/fc2 operands

    d_x = nc.dram_tensor("x", [BL, H, W, C], F32, kind="ExternalInput")
    d_wq = nc.dram_tensor("wq", [128, 2, C], BF16, kind="ExternalInput")
    d_wk = nc.dram_tensor("wk", [128, 2, C], BF16, kind="ExternalInput")
    d_wv = nc.dram_tensor("wv", [128, 2, C], BF16, kind="ExternalInput")
    d_wp = nc.dram_tensor("wp", [128, 2, C], BF16, kind="ExternalInput")
    d_w1 = nc.dram_tensor("w1", [128, 2, HID], MDT, kind="ExternalInput")
    d_w2 = nc.dram_tensor("w2", [128, 8, C], MDT, kind="ExternalInput")
    d_bq = nc.dram_tensor("bq", [128, 2], F32, kind="ExternalInput")
    d_bk = nc.dram_tensor("bk", [128, 2], F32, kind="ExternalInput")
    d_b1 = nc.dram_tensor("b1", [128, 8], F32, kind="ExternalInput")
    d_expb = nc.dram_tensor("expb", [128, 4, 2, 98], BF16, kind="ExternalInput")
    d_out = nc.dram_tensor("out", [BL, H, W, C], F32, kind="ExternalOutput")

    # one macro = one (image, window-row): [98 tokens(w,r,t), 4 chunks, C]
    xw6 = d_x.rearrange(
        "b (wr r) (ck w t) ch -> (b wr) r ck w t ch", r=WS, ck=NCHUNK, w=2, t=WS
    )
    ow6 = d_out.rearrange(
        "b (wr r) (ck w t) ch -> (b wr) r ck w t ch", r=WS, ck=NCHUNK, w=2, t=WS
    )

    def xv(mt, w, ck):
        # [7r, 7t, 256ch]; (t, ch) merge -> 2-dim DRAM AP
        return xw6[mt][:, ck, w, :, :]

    def ov(mt, w, ck):
        return ow6[mt][:, ck, w, :, :]

    from contextlib import ExitStack

    with tile.TileContext(nc) as tc:
        with ExitStack() as octx:
            resident = octx.enter_context(tc.tile_pool(name="res", bufs=1))
            # ------------- residents
            wq_sb = resident.tile([128, 2, C], BF16)
            nc.sync.dma_start(wq_sb, d_wq[:])
            wk_sb = resident.tile([128, 2, C], BF16)
            nc.sync.dma_start(wk_sb, d_wk[:])
            wv_sb = resident.tile([128, 2, C], BF16)
            nc.sync.dma_start(wv_sb, d_wv[:])
            wp_sb = resident.tile([128, 2, C], BF16)
            nc.sync.dma_start(wp_sb, d_wp[:])
            w1_sb = resident.tile([128, 2, HID], MDT)
            nc.sync.dma_start(w1_sb, d_w1[:])
            w2_sb = resident.tile([128, 8, C], MDT)
            nc.sync.dma_start(w2_sb, d_w2[:])
            bq_sb = resident.tile([128, 2], F32)
            nc.sync.dma_start(bq_sb, d_bq[:])
            bk_sb = resident.tile([128, 2], F32)
            nc.sync.dma_start(bk_sb, d_bk[:])
            b1_sb = resident.tile([128, 8], F32)
            nc.sync.dma_start(b1_sb, d_b1[:])
            expb_sb = resident.tile([128, 4, 2, 98], BF16)
            nc.sync.dma_start(expb_sb, d_expb[:])
            ident = resident.tile([128, 128], BF16)
            make_identity(nc, ident)
            eps_sb = resident.tile([128, 1], F32)
            nc.vector.memset(eps_sb, EPS)

            # big residents: x (becomes x2 in place), LN stats
            x_res = resident.tile([98, TOT_CK, C], BF16)
            mv1 = resident.tile([98, TOT_CK, 2], F32)
            rstd1 = resident.tile([98, TOT_CK], F32)
            mv2 = resident.tile([98, TOT_CK, 2], F32)
            rstd2 = resident.tile([98, TOT_CK], F32)

            # =========================== PREPASS ===========================
            with ExitStack() as ctx:
                xin = ctx.enter_context(tc.tile_pool(name="xin", bufs=3))
                stp = ctx.enter_context(tc.tile_pool(name="stp", bufs=4))
                for mt in range(N_MACRO):
                    xst = xin.tile([98, NCHUNK, 264], F32, tag="xst")
                    for ck in range(NCHUNK):
                        for w in range(2):
                            nc.sync.dma_start(
                                xst[w * 49 : (w + 1) * 49, ck, 0:C], xv(mt, w, ck)
                            )
                    # cast to resident bf16 (scalar engine, table-free Copy)
                    nc.scalar.activation(
                        x_res[:, mt * 4 : (mt + 1) * 4, :],
                        xst[:, :, 0:C],
                        AF.Copy,
                    )
                    for ck in range(NCHUNK):
                        st = stp.tile([98, 6], F32, tag="st")
                        nc.vector.bn_stats(st, xst[:, ck, 0:C])
                        nc.vector.bn_aggr(mv1[:, mt * 4 + ck, :], st)
                    if mt % 8 == 7:
                        g0 = (mt - 7) * 4
                        sd1 = stp.tile([98, 32], F32, tag="sd1")
                        nc.scalar.activation(
                            sd1, mv1[:, g0 : g0 + 32, 1], AF.Sqrt, bias=eps_sb[:98]
                        )
                        nc.vector.reciprocal(rstd1[:, g0 : g0 + 32], sd1)

            # =========================== PASS A ============================
            with ExitStack() as ctx:
                xhp = ctx.enter_context(tc.tile_pool(name="xhp", bufs=4))
                hTp = ctx.enter_context(tc.tile_pool(name="hTp", bufs=2))
                qTp = ctx.enter_context(tc.tile_pool(name="qTp", bufs=2))
                kTp = ctx.enter_context(tc.tile_pool(name="kTp", bufs=2))
                vap = ctx.enter_context(tc.tile_pool(name="vap", bufs=8))
                esp = ctx.enter_context(tc.tile_pool(name="esp", bufs=4))
                atp = ctx.enter_context(tc.tile_pool(name="atp", bufs=4))
                aTp = ctx.enter_context(tc.tile_pool(name="aTp", bufs=2))
                sta = ctx.enter_context(tc.tile_pool(name="sta", bufs=4))
                ps_sc = ctx.enter_context(
                    tc.tile_pool(name="ps_sc", bufs=1, space="PSUM")
                )
                ps_av = ctx.enter_context(
                    tc.tile_pool(name="ps_av", bufs=1, space="PSUM")
                )
                ps_mm = ctx.enter_context(
                    tc.tile_pool(name="ps_mm", bufs=1, space="PSUM")
                )
                ps_tr = ctx.enter_context(
                    tc.tile_pool(name="ps_tr", bufs=1, space="PSUM")
                )

                for mt in range(N_MACRO):
                    c0 = mt * 4
                    # ---- LN1 apply + transpose -> hT [128, 2, 392] bf16
                    hT = hTp.tile([128, 2, NT], BF16, tag="hT")
                    for ck in range(NCHUNK):
                        xh = xhp.tile([98, C], BF16, tag="xh")
                        nc.vector.tensor_scalar(
                            xh,
                            x_res[:, c0 + ck, :],
                            mv1[:, c0 + ck, 0:1],
                            rstd1[:, c0 + ck : c0 + ck + 1],
                            ALU.subtract,
                            ALU.mult,
                        )
                        pst = ps_tr.tile(
                            [128, 2, 98], BF16, tag="pst",
                            padded_shape=[128, 2, 1024],
                        )
                        for kc in range(2):
                            nc.tensor.transpose(
                                pst[:, kc, :], xh[:, kc * 128 : (kc + 1) * 128],
                                ident[:98, :98],
                            )
                        nc.scalar.copy(hT[:, :, ck * 98 : (ck + 1) * 98], pst)

                    # ---- q, k channel-major
                    qT = qTp.tile([128, 2, NT], BF16, tag="qT")
                    kT = kTp.tile([128, 2, NCHUNK, 128], BF16, tag="kT")
                    if mt < 2:  # zero pad key slots once per buffer
                        nc.vector.memset(kT, 0.0)
                    for g in range(2):
                        psq = ps_mm.tile([128, NT], F32, tag="mm")
                        for kc in range(2):
                            nc.tensor.matmul(
                                psq,
                                lhsT=wq_sb[:, kc, g * 128 : (g + 1) * 128],
                                rhs=hT[:, kc, :],
                                start=(kc == 0),
                                stop=(kc == 1),
                            )
                        if has_bq:
                            nc.scalar.activation(
                                qT[:, g, :], psq, AF.Identity,
                                bias=bq_sb[:, g : g + 1],
                            )
                        else:
                            nc.scalar.copy(qT[:, g, :], psq)
                        psk = ps_mm.tile([128, NT], F32, tag="mm")
                        for kc in range(2):
                            nc.tensor.matmul(
                                psk,
                                lhsT=wk_sb[:, kc, g * 128 : (g + 1) * 128],
                                rhs=hT[:, kc, :],
                                start=(kc == 0),
                                stop=(kc == 1),
                            )
                        pskv = psk.rearrange("p (ck wt) -> p ck wt", ck=NCHUNK)
                        for w in range(2):
                            dst = kT[:, g, :, w * 64 : w * 64 + 49]
                            src = pskv[:, :, w * 49 : (w + 1) * 49]
                            if has_bq:
                                nc.vector.tensor_scalar_add(
                                    dst, src, bk_sb[:, g : g + 1]
                                )
                            else:
                                nc.vector.tensor_copy(dst, src)

                    # ---- per chunk: v, scores, exp*bias, AV, normalize
                    a_tms = []
                    for ck in range(NCHUNK):
                        # v token-major -> va [128 jslot, 8, 33]
                        psv = ps_mm.tile([128, C], F32, tag="mm", name="psv")
                        for w in range(2):
                            t0 = ck * 98 + w * 49
                            for kc in range(2):
                                nc.tensor.matmul(
                                    psv[w * 64 : w * 64 + 49, :],
                                    lhsT=hT[:, kc, t0 : t0 + 49],
                                    rhs=wv_sb[:, kc, :],
                                    start=(kc == 0),
                                    stop=(kc == 1),
                                )
                        va = vap.tile([128, NH, HD + 1], BF16, tag="va")
                        if mt < 2:  # preset ones column + zero pad rows, per slot
                            nc.gpsimd.memset(va, 0.0)
                            nc.gpsimd.memset(va[:, :, HD : HD + 1], 1.0)
                        psvv = psv.rearrange("p (h d) -> p h d", h=NH)
                        nc.vector.tensor_copy(va[0:49, :, 0:HD], psvv[0:49])
                        nc.vector.tensor_copy(va[64:113, :, 0:HD], psvv[64:113])

                        # scores: [128, 4r, 2*98] psum (4 banks), K=32 packed
                        pss = ps_sc.tile(
                            [128, 4, 196], F32, tag="sc", padded_shape=[128, 4, 512]
                        )
                        for r in range(4):
                            for g in range(2):
                                nc.tensor.matmul(
                                    pss[:, r, g * 98 : (g + 1) * 98],
                                    lhsT=kT[r * 32 : (r + 1) * 32, g, ck, :],
                                    rhs=qT[r * 32 : (r + 1) * 32, g, ck * 98 : ck * 98 + 98],
                                    start=True,
                                    stop=True,
                                    tile_position=(r * 32, 0),
                                    skip_group_check=True,
                                )
                        # es block j holds head r=(j%2)*2+j//2 (psum bank/slot order)
                        es = esp.tile([128, 4, 2, 98], BF16, tag="es")
                        es_v = es.rearrange("p r g i -> p r (g i)")
                        nc.scalar.activation(es_v, pss, AF.Exp)
                        # multiplicative rel-pos bias; zeroes pad keys and
                        # cross-window blocks
                        es_eng = nc.gpsimd if (ck % 2 == 1) else nc.vector
                        es_eng.tensor_tensor(es, es, expb_sb, ALU.mult)

                        # AV: one matmul per head, K=128 covers both windows
                        psav = ps_av.tile([98, NH, HD + 1], F32, tag="av")
                        for h in range(NH):
                            r, g = h % 4, h // 4
                            nc.tensor.matmul(
                                psav[:, h, :],
                                lhsT=es[:, r, g, :],
                                rhs=va[:, h, :],
                                start=True,
                                stop=True,
                            )
                        rd = sta.tile([98, NH, 1], F32, tag="rd")
                        nc.vector.reciprocal(rd, psav[:, :, HD : HD + 1])
                        a_tm = atp.tile([98, NH, HD], BF16, tag="atm")
                        nc.vector.tensor_tensor(
                            a_tm,
                            psav[:, :, 0:HD],
                            rd.to_broadcast([98, NH, HD]),
                            ALU.mult,
                        )
                        a_tms.append(a_tm)

                    # ---- transpose attn out, proj, residual, LN2 stats
                    aT = aTp.tile([128, 2, NT], BF16, tag="aT")
                    for ck in range(NCHUNK):
                        af = a_tms[ck].rearrange("p h d -> p (h d)")
                        psat = ps_tr.tile(
                            [128, 2, 98], BF16, tag="pst",
                            padded_shape=[128, 2, 1024],
                        )
                        for kc in range(2):
                            nc.tensor.transpose(
                                psat[:, kc, :], af[:, kc * 128 : (kc + 1) * 128],
                                ident[:98, :98],
                            )
                        nc.scalar.copy(aT[:, :, ck * 98 : (ck + 1) * 98], psat)
                    for ck in range(NCHUNK):
                        psp = ps_mm.tile([98, C], F32, tag="mm", name="psp")
                        for kc in range(2):
                            nc.tensor.matmul(
                                psp,
                                lhsT=aT[:, kc, ck * 98 : (ck + 1) * 98],
                                rhs=wp_sb[:, kc, :],
                                start=(kc == 0),
                                stop=(kc == 1),
                            )
                        # x2 = x + proj(attn)  (in place into x_res)
                        nc.vector.tensor_tensor(
                            x_res[:, c0 + ck, :], psp, x_res[:, c0 + ck, :], ALU.add
                        )
                        st2 = sta.tile([98, 6], F32, tag="st2")
                        nc.vector.bn_stats(st2, x_res[:, c0 + ck, :])
                        nc.vector.bn_aggr(mv2[:, c0 + ck, :], st2)
                    if mt % 8 == 7:
                        g0 = (mt - 7) * 4
                        sd2 = sta.tile([98, 32], F32, tag="sd2")
                        nc.scalar.activation(
                            sd2, mv2[:, g0 : g0 + 32, 1], AF.Sqrt, bias=eps_sb[:98]
                        )
                        nc.vector.reciprocal(rstd2[:, g0 : g0 + 32], sd2)

            # =========================== PASS B ============================
            with ExitStack() as ctx:
                xh2p = ctx.enter_context(tc.tile_pool(name="xh2p", bufs=4))
                h2Tp = ctx.enter_context(tc.tile_pool(name="h2Tp", bufs=2))
                gTp = ctx.enter_context(tc.tile_pool(name="gTp", bufs=2))
                otp = ctx.enter_context(tc.tile_pool(name="otp", bufs=2))
                ps_f1 = ctx.enter_context(
                    tc.tile_pool(name="ps_f1", bufs=1, space="PSUM")
                )
                ps_f2 = ctx.enter_context(
                    tc.tile_pool(name="ps_f2", bufs=2, space="PSUM")
                )
                ps_t2 = ctx.enter_context(
                    tc.tile_pool(name="ps_t2", bufs=1, space="PSUM")
                )

                for mt in range(N_MACRO):
                    c0 = mt * 4
                    h2T = h2Tp.tile([128, 2, NT], MDT, tag="h2T")
                    for ck in range(NCHUNK):
                        xh2 = xh2p.tile([98, C], BF16, tag="xh2")
                        nc.vector.tensor_scalar(
                            xh2,
                            x_res[:, c0 + ck, :],
                            mv2[:, c0 + ck, 0:1],
                            rstd2[:, c0 + ck : c0 + ck + 1],
                            ALU.subtract,
                            ALU.mult,
                        )
                        pst2 = ps_t2.tile(
                            [128, 2, 98], BF16, tag="pst2",
                            padded_shape=[128, 2, 1024],
                        )
                        for kc in range(2):
                            nc.tensor.transpose(
                                pst2[:, kc, :], xh2[:, kc * 128 : (kc + 1) * 128],
                                ident[:98, :98],
                            )
                        nc.vector.tensor_copy(
                            h2T[:, :, ck * 98 : (ck + 1) * 98], pst2
                        )

                    # fc1 + gelu -> gT [128, 8(mc), 392]
                    gT = gTp.tile([128, 2, 4, NT], MDT, tag="gT")
                    for mq in range(2):
                        psf = ps_f1.tile(
                            [128, 4, NT], F32, tag="f1", padded_shape=[128, 4, 512]
                        )
                        for sub in range(4):
                            mc = mq * 4 + sub
                            if use_fp8:
                                nc.tensor.matmul(
                                    psf[:, sub, :],
                                    lhsT=w1_sb[:, :, mc * 128 : (mc + 1) * 128],
                                    rhs=h2T,
                                    start=True,
                                    stop=True,
                                    perf_mode=DR,
                                )
                            else:
                                for kc in range(2):
                                    nc.tensor.matmul(
                                        psf[:, sub, :],
                                        lhsT=w1_sb[:, kc, mc * 128 : (mc + 1) * 128],
                                        rhs=h2T[:, kc, :],
                                        start=(kc == 0),
                                        stop=(kc == 1),
                                    )
                        inv = (1.0 / WSCALE) if use_fp8 else 1.0
                        if has_b1:
                            # per-mc bias varies along free dim -> per-mc ops
                            for sub in range(4):
                                nc.scalar.activation(
                                    gT[:, mq, sub, :],
                                    psf[:, sub, :],
                                    AF.Gelu,
                                    bias=b1_sb[:, mq * 4 + sub : mq * 4 + sub + 1],
                                    scale=inv,
                                )
                        else:
                            nc.scalar.activation(gT[:, mq], psf, AF.Gelu, scale=inv)

                    # fc2 + residual -> out
                    o_t = otp.tile([98, NCHUNK, 264], F32, tag="ot")
                    for ck in range(NCHUNK):
                        ps2 = ps_f2.tile([98, C], F32, tag="f2")
                        if use_fp8:
                            for pr in range(4):
                                nc.tensor.matmul(
                                    ps2,
                                    lhsT=gT[:, pr // 2, (pr % 2) * 2 : (pr % 2) * 2 + 2,
                                            ck * 98 : (ck + 1) * 98],
                                    rhs=w2_sb[:, 2 * pr : 2 * pr + 2, :],
                                    start=(pr == 0),
                                    stop=(pr == 3),
                                    perf_mode=DR,
                                )
                        else:
                            for hc in range(8):
                                nc.tensor.matmul(
                                    ps2,
                                    lhsT=gT[:, hc // 4, hc % 4, ck * 98 : (ck + 1) * 98],
                                    rhs=w2_sb[:, hc, :],
                                    start=(hc == 0),
                                    stop=(hc == 7),
                                )
                        if use_fp8:
                            nc.vector.scalar_tensor_tensor(
                                o_t[:, ck, 0:C],
                                ps2,
                                1.0 / WSCALE,
                                x_res[:, c0 + ck, :],
                                ALU.mult,
                                ALU.add,
                            )
                        else:
                            nc.vector.tensor_tensor(
                                o_t[:, ck, 0:C], ps2, x_res[:, c0 + ck, :], ALU.add
                            )
                    for ck in range(NCHUNK):
                        for w in range(2):
                            nc.sync.dma_start(
                                ov(mt, w, ck), o_t[w * 49 : (w + 1) * 49, ck, 0:C]
                            )

    _split_multi_waits(nc)
    return nc


# ------------------------------------------------------------- host wrapper
_PROGRAM_CACHE = {}


def _prep_weights(norm1_g, norm1_b, qkv_w, qkv_b, bias_table, proj_w, proj_b,
                  norm2_g, norm2_b, fc1_w, fc1_b, fc2_w, fc2_b, use_fp8=USE_FP8):
    f32 = np.float32
    bf16 = ml_dtypes.bfloat16
    fp8 = ml_dtypes.float8_e4m3fn
    # fold LN1 affine into qkv weights
    wqkv = (norm1_g[:, None] * qkv_w).astype(f32)  # [C, 3C]
    bqkv = (norm1_b @ qkv_w + qkv_b).astype(f32)  # [3C]
    wq = wqkv[:, 0:C] * SCALE
    bq = bqkv[0:C] * SCALE
    wk = wqkv[:, C : 2 * C]
    bk = bqkv[C : 2 * C]
    wv = wqkv[:, 2 * C : 3 * C]
    bv = bqkv[2 * C : 3 * C]
    # fold LN2 affine into fc1
    w1 = (norm2_g[:, None] * fc1_w).astype(f32)  # [C, HID]
    b1 = (norm2_b @ fc1_w + fc1_b).astype(f32)  # [HID]

    def kpart(w):  # [K, O] -> [128, K//128, O]
        k, o = w.shape
        return np.ascontiguousarray(
            w.reshape(k // 128, 128, o).transpose(1, 0, 2)
        )

    mdt = fp8 if use_fp8 else bf16
    ws = WSCALE if use_fp8 else 1.0
    arrs = {
        "wq": kpart(wq).astype(bf16),
        "wk": kpart(wk).astype(bf16),
        "wv": kpart(wv).astype(bf16),
        "wp": kpart(proj_w.astype(f32)).astype(bf16),
        "w1": kpart(w1 * ws).astype(mdt),
        "w2": kpart(fc2_w.astype(f32) * ws).astype(mdt),
        "bq": np.ascontiguousarray(bq.reshape(2, 128).T).astype(f32),
        "bk": np.ascontiguousarray(bk.reshape(2, 128).T).astype(f32),
        "b1": np.ascontiguousarray((b1 / 1.0).reshape(8, 128).T).astype(f32),
    }
    # multiplicative rel-pos bias table: expb[j, r, g, i]; zero on pad key
    # rows and cross-window blocks
    bias_full = np.asarray(bias_table)[REL_IDX]  # [i, j, NH]
    eb = np.zeros((128, 4, 2, 98), dtype=f32)
    for h in range(NH):
        r, g = h % 4, h // 4
        bj = np.exp(bias_full[:, :, h].T.astype(f32))  # [j, i]
        eb[0:49, r, g, 0:49] = bj
        eb[64:113, r, g, 49:98] = bj
    arrs["expb"] = eb.astype(bf16)

    for name, v in (("bv", bv), ("bp", proj_b), ("b2", fc2_b)):
        assert np.abs(v).max() < 1e-30, f"nonzero {name} not supported yet"
    has_bq = bool(np.abs(bq).max() > 0 or np.abs(bk).max() > 0)
    has_b1 = bool(np.abs(b1).max() > 0)
    return arrs, has_bq, has_b1


def kernel(**inputs):
    x = np.asarray(inputs["x"], dtype=np.float32)
    prep, has_bq, has_b1 = _prep_weights(
        np.asarray(inputs["norm1_g"], np.float32),
        np.asarray(inputs["norm1_b"], np.float32),
        np.asarray(inputs["qkv_w"], np.float32),
        np.asarray(inputs["qkv_b"], np.float32),
        np.asarray(inputs["bias_table"], np.float32),
        np.asarray(inputs["proj_w"], np.float32),
        np.asarray(inputs["proj_b"], np.float32),
        np.asarray(inputs["norm2_g"], np.float32),
        np.asarray(inputs["norm2_b"], np.float32),
        np.asarray(inputs["fc1_w"], np.float32),
        np.asarray(inputs["fc1_b"], np.float32),
        np.asarray(inputs["fc2_w"], np.float32),
        np.asarray(inputs["fc2_b"], np.float32),
    )

    key = ("nc", USE_FP8, has_bq, has_b1)
    if key not in _PROGRAM_CACHE:
        _PROGRAM_CACHE[key] = build_program(USE_FP8, has_bq, has_b1)
        _PROGRAM_CACHE["nc"] = _PROGRAM_CACHE[key]
    nc = _PROGRAM_CACHE[key]

    in_maps = []
    for c in range(N_CORES):
        m = dict(prep)
        m["x"] = np.ascontiguousarray(x[c * BL : (c + 1) * BL])
        in_maps.append(m)

    res = run_bass_kernel_spmd(nc, in_maps, core_ids=list(range(N_CORES)))
    out = np.concatenate([res.results[c]["out"] for c in range(N_CORES)], axis=0)
    return out.astype(np.float32)
